# revision 5
# baseline (speedup 1.0000x reference)
"""AdaptiveSparseAttention Trainium2 kernel (8-core head-parallel), v2.

Problem: B=1, H=16, S=2048, D=128 fp32, causal attention with an adaptive
block mask: mean-pool Q/K per 64-block, softmax block scores, keep the
minimal top-p (0.95) set of key blocks per query block (plus diagonal).

Sharding: 2 heads per NeuronCore, fully local (no collectives).

v2 changes vs baseline (106.5us):
  - per-head DMA rings (h0 on sync HWDGE, h1 on scalar HWDGE), k first,
    then the first v quarter, then q, then the v tail - so the mask
    chain and first flash waves start ~14us instead of ~45us.
  - no DVE pre-casts of k/q: PE transposes the f32 naturals directly
    (f32 transpose -> f32 psum), the psum->SBUF copy on ScalarE does
    the bf16 cast.
  - block sums for the mask via PE matmuls against a [128,2] block
    indicator (rhs), accumulated from the f32 naturals - replaces the
    slow DVE reduces and removes the cast dependency.
  - v layout copies on DVE (gpsimd was ~0.3 elem/cy and serialized the
    whole preamble behind its queue).
  - negk for both heads packed into one [128,S] tile (h0 rows 0-31,
    h1 rows 32-63, rows 64-127 zeroed once); two indall constants
    select the right band, so the per-head [96,S] memsets are gone.
  - main loop in waves of 4 chunks ([128,1024] f32 psum tiles): one
    exp per wave amortizes ScalarE's ~352-cycle per-instruction
    overhead (exp is the second-tightest engine after PE).
  - software-pipelined PV: wave w's PV matmuls issue after wave w+1's
    QK/mask matmuls, so the tensor queue never stalls on the exp.
  - outputs staged per (head, group of 256 rows) and written with one
    128KB DMA on the head's ring.
"""

import math
import threading

import numpy as np

_B, _H, _S, _D = 1, 16, 2048, 128
_NCORES = 8
_HLOC = _H // _NCORES  # heads per core
_BLK = 64
_NB = _S // _BLK       # 32 key/query blocks
_TAU = 0.95
_SCALE = 1.0 / math.sqrt(_D)
_SHIFT = 9.0           # constant softmax shift; |scaled logits| < ~6
_BIGM = 1.0e9          # additive mask magnitude (pre-scale)
_NEG_BL = -1.0e30      # block-logit causal mask value (matches reference)

_NCHUNK = _S // 128    # 16 sequence chunks of 128
_NGRP = _S // 256      # 8 query groups of 256
_WAVE = 4              # kj chunks per LT wave


class _Head:
    pass


def _emit(nc, tc, pools, consts, q_d, k_d, v_d, out_d, mybir):
    f32 = mybir.dt.float32
    bf16 = mybir.dt.bfloat16
    AF = mybir.ActivationFunctionType
    OP = mybir.AluOpType
    AX = mybir.AxisListType

    natp = pools["natp"]
    psM = pools["psM"]
    psP = pools["psP"]
    big = pools["big"]
    sm = pools["sm"]
    ptp = pools["ptp"]
    outp = pools["outp"]

    identf = consts["identf"]
    indall = consts["indall"]  # [indall_h0, indall_h1]
    tri128 = consts["tri128"]
    causal_add = consts["causal_add"]
    causal01 = consts["causal01"]
    eye01 = consts["eye01"]
    nshift = consts["nshift"]
    blockind = consts["blockind"]
    negk = consts["negk"]      # shared [128, S] bf16, rows 64-127 zero

    rings = [nc.sync, nc.scalar]

    heads = []
    for h in range(_HLOC):
        H = _Head()
        H.h = h
        H.ring = rings[h]
        H.ind = indall[h]
        H.qT = big.tile([128, _S], bf16, tag="qT", name=f"qT{h}")
        H.kT = big.tile([128, _S], bf16, tag="kT", name=f"kT{h}")
        H.vb = big.tile([128, _NCHUNK * 129], bf16, tag="vb", name=f"vb{h}")
        H.vb3 = H.vb[:].rearrange("p (c x) -> p c x", x=129)
        H.knat = natp.tile([128, _S], f32, tag="knat", name=f"knat{h}")
        H.qnat = natp.tile([128, _S], f32, tag="qnat", name=f"qnat{h}")
        H.vnat = natp.tile([128, _S], f32, tag="vnat", name=f"vnat{h}")
        heads.append(H)

    # ---- stage A: DMAs.  Each head owns one HWDGE ring; k first (mask
    # + first waves), then v chunks 0-3 (first PVs), then q, then the v
    # tail.  Halves/quarters keep the per-dma fixed cost amortized while
    # letting the PE start transposing early.
    def dma_nat(H, nat, src_d, lo, n):
        H.ring.dma_start(
            nat[:, lo * 128:(lo + n) * 128].rearrange("p (c d) -> p c d",
                                                      d=128),
            src_d[H.h, lo * 128:(lo + n) * 128, :].rearrange(
                "(c p) d -> p c d", p=128))

    for H in heads:
        dma_nat(H, H.knat, k_d, 0, 8)
        dma_nat(H, H.knat, k_d, 8, 8)
    for H in heads:
        dma_nat(H, H.vnat, v_d, 0, 4)
    for H in heads:
        dma_nat(H, H.qnat, q_d, 0, 8)
        dma_nat(H, H.qnat, q_d, 8, 8)
    for H in heads:
        for quarter in range(1, 4):
            dma_nat(H, H.vnat, v_d, quarter * 4, 4)

    # ---- stage B: PE transposes (f32 in -> f32 psum, cast to bf16 in
    # the ScalarE psum->SBUF copy) + block-sum matmuls vs blockind.
    # bsumps cols: h*64 + 0:32 = k block sums, h*64 + 32:64 = q.
    bsumps = psM.tile([128, 128], f32, tag="acc", name="bsumps")
    for H in heads:
        h = H.h
        for tname, nat, dst in (("k", H.knat, H.kT), ("q", H.qnat, H.qT)):
            base = h * 64 + (0 if tname == "k" else 32)
            for half in range(2):
                pack = psP.tile([128, 1024], f32, tag="lt",
                                name=f"pack{h}{tname}{half}")
                for j in range(8):
                    c = half * 8 + j
                    nc.tensor.transpose(pack[:, j * 128:(j + 1) * 128],
                                        nat[:, c * 128:(c + 1) * 128],
                                        identf[:])
                    nc.tensor.matmul(bsumps[:, base + 2 * c:base + 2 * c + 2],
                                     nat[:, c * 128:(c + 1) * 128],
                                     blockind[:], start=True, stop=True)
                lo = half * 1024
                nc.scalar.copy(dst[:, lo:lo + 1024], pack[:])
        H.bsum_sb = sm.tile([128, 64], f32, tag="bsum", name=f"bsum{h}")
        nc.scalar.copy(H.bsum_sb[:], bsumps[:, h * 64:h * 64 + 64])

    # ---- stage C: v layout casts (DVE) for chunks 0-3; the rest are
    # emitted interleaved into the main loop (paced by the v DMAs).
    def vb_quarter(H, quarter):
        lo = quarter * 4
        nc.vector.tensor_copy(
            H.vb3[:, lo:lo + 4, 0:128],
            H.vnat[:, lo * 128:(lo + 4) * 128].rearrange(
                "p (c d) -> p c d", d=128))
        nc.vector.memset(H.vb3[:, lo:lo + 4, 128], 1.0)

    for H in heads:
        vb_quarter(H, 0)

    # ---- stage D: block-score keep mask per head -> negk band.
    for H in heads:
        h = H.h
        kbT = H.bsum_sb[:, 0:32]
        qbT = H.bsum_sb[:, 32:64]
        ksum = sm.tile([128, 1], f32, tag="ksum", name=f"ksum{h}")
        nc.vector.reduce_sum(ksum[:], kbT, axis=AX.X)
        mean64 = sm.tile([128, 1], f32, tag="mean64", name=f"mean64{h}")
        nc.vector.tensor_scalar_mul(mean64[:], ksum[:], 1.0 / float(_NB))
        kbs = sm.tile([128, _NB], f32, tag="kbs", name=f"kbs{h}")
        nc.vector.tensor_scalar_sub(kbs[:], kbT, mean64[:])

        blp = psM.tile([32, 32], f32, tag="acc", name=f"blp{h}")
        nc.tensor.matmul(blp[:], qbT, kbs[:], start=True, stop=True)
        bl = sm.tile([32, 32], f32, tag="bl", name=f"bl{h}")
        nc.vector.scalar_tensor_tensor(
            bl[:], blp[:], _SCALE / float(_BLK * _BLK), causal_add[:],
            op0=OP.mult, op1=OP.add)
        mx = sm.tile([32, 1], f32, tag="mx", name=f"mx{h}")
        nc.vector.reduce_max(mx[:], bl[:], axis=AX.X)
        nmx = sm.tile([32, 1], f32, tag="nmx", name=f"nmx{h}")
        nc.vector.tensor_scalar_mul(nmx[:], mx[:], -1.0)
        # unnormalized block softmax: keep test compares the sum of
        # strictly-greater exps against tau * rowsum
        bp = sm.tile([32, 32], f32, tag="bp", name=f"bp{h}")
        rs = sm.tile([32, 1], f32, tag="rs", name=f"rs{h}")
        nc.scalar.activation(bp[:], bl[:], AF.Exp, bias=nmx[:], scale=1.0,
                             accum_out=rs[:])
        taurs = sm.tile([32, 1], f32, tag="taurs", name=f"taurs{h}")
        nc.vector.tensor_scalar_mul(taurs[:], rs[:], _TAU)

        a_ap = bp[:].unsqueeze(1).broadcast_to((32, 32, 32))
        b_ap = bp[:].unsqueeze(2).broadcast_to((32, 32, 32))
        gt = sm.tile([32, 32 * 32], f32, tag="gt", name=f"gt{h}")
        gt3 = gt[:].rearrange("p (a b) -> p a b", a=32)
        nc.vector.tensor_tensor(gt3, a_ap, b_ap, op=OP.is_gt)
        pr = sm.tile([32, 32 * 32], f32, tag="pr", name=f"pr{h}")
        pr3 = pr[:].rearrange("p (a b) -> p a b", a=32)
        nc.vector.tensor_tensor(pr3, gt3, a_ap, op=OP.mult)
        tt = sm.tile([32, 32], f32, tag="tt", name=f"tt{h}")
        nc.vector.reduce_sum(tt[:], pr3, axis=AX.X)
        keep = sm.tile([32, 32], f32, tag="keep", name=f"keep{h}")
        nc.vector.scalar_tensor_tensor(
            keep[:], tt[:], taurs[:], causal01[:], op0=OP.is_lt, op1=OP.mult)
        nc.vector.tensor_tensor(keep[:], keep[:], eye01[:], op=OP.max)
        keepT = sm.tile([32, 32], f32, tag="keepT", name=f"keepT{h}")
        nc.vector.transpose(keepT[:], keep[:])
        nc.vector.tensor_scalar(
            negk[h * 32:(h + 1) * 32, :].rearrange("p (a b) -> p a b",
                                                   b=_BLK),
            keepT[:].unsqueeze(2).broadcast_to((32, 32, _BLK)),
            1.0, _BIGM, op0=OP.subtract, op1=OP.mult)

    # ---- stage E: main flash loop.  Waves of 4 kj chunks into
    # [128,1024] f32 psum tiles; heads interleaved per wave; PV of wave
    # w issued after the QK/mask matmuls of wave w+1 (software pipeline)
    # so PE never stalls on the exp.
    pending = []   # list of (H, g, w0, wn, ptw, acc) awaiting PV issue
    done_groups = []  # (H, g, acc) whose last wave PVs have been issued

    def flush_pending():
        for H, g, w0, wn, ptw, acc in pending:
            nchunks = 2 * g + 2
            for ci in range(w0, w0 + wn):
                for t in range(max(2 * g, ci), 2 * g + 2):
                    nc.tensor.matmul(
                        acc[t - 2 * g][:],
                        ptw[:, (ci - w0) * 256 + (t - 2 * g) * 128:
                            (ci - w0) * 256 + (t - 2 * g) * 128 + 128],
                        H.vb3[:, ci, :],
                        start=(ci == 0), stop=(ci == t))
            if w0 + wn == nchunks:
                done_groups.append((H, g, acc))
        pending.clear()

    def finalize_done():
        for H, g, acc in done_groups:
            h = H.h
            qlo = g * 256
            rden0 = sm.tile([128, 1], f32, tag="rden", name=f"rd0_{h}_{g}")
            rden1 = sm.tile([128, 1], f32, tag="rden", name=f"rd1_{h}_{g}")
            nc.vector.reciprocal(rden0[:], acc[0][:, 128:129])
            nc.vector.reciprocal(rden1[:], acc[1][:, 128:129])
            o = outp.tile([128, 256], f32, tag="o", name=f"o{h}_{g}")
            nc.vector.tensor_scalar_mul(o[:, 0:128], acc[0][:, 0:128],
                                        rden0[:])
            nc.vector.tensor_scalar_mul(o[:, 128:256], acc[1][:, 0:128],
                                        rden1[:])
            H.ring.dma_start(
                out_d[h, qlo:qlo + 256, :].rearrange("(t p) d -> p t d",
                                                     p=128),
                o[:].rearrange("p (t d) -> p t d", d=128))
        done_groups.clear()

    for g in range(_NGRP):
        nchunks = 2 * g + 2
        for H in heads:
            H.acc = [psM.tile([128, 129], f32, tag="acc",
                              name=f"acc{H.h}_{g}_{t}") for t in range(2)]
        for w0 in range(0, nchunks, _WAVE):
            wn = min(_WAVE, nchunks - w0)
            new_work = []
            for H in heads:
                qlo = g * 256
                ltw = psP.tile([128, 1024], f32, tag="lt",
                               name=f"lt{H.h}_{g}_{w0}")
                for ci in range(w0, w0 + wn):
                    sl = ltw[:, (ci - w0) * 256:(ci - w0) * 256 + 256]
                    nc.tensor.matmul(sl, H.kT[:, ci * 128:(ci + 1) * 128],
                                     H.qT[:, qlo:qlo + 256],
                                     start=True, stop=False)
                    nc.tensor.matmul(sl, H.ind[:, ci * 128:(ci + 1) * 128],
                                     negk[:, qlo:qlo + 256],
                                     start=False, stop=True)
                    if ci >= 2 * g:  # diagonal 128-band token causal mask
                        off = (ci - w0) * 256 + (ci - 2 * g) * 128
                        nc.vector.tensor_tensor(
                            ltw[:, off:off + 128], ltw[:, off:off + 128],
                            tri128[:], op=OP.add)
                ptw = ptp.tile([128, 1024], bf16, tag="pt",
                               name=f"pt{H.h}_{g}_{w0}")
                nc.scalar.activation(ptw[:, 0:wn * 256], ltw[:, 0:wn * 256],
                                     AF.Exp, bias=nshift[:], scale=_SCALE)
                new_work.append((H, g, w0, wn, ptw, H.acc))
            flush_pending()
            pending.extend(new_work)
            finalize_done()
        # v cast tail, paced by the v DMAs; group g+1 needs chunks
        # <= 2g+3, quarter q covers chunks 4q..4q+3
        if g in (0, 1, 2):
            for H in heads:
                vb_quarter(H, g + 1)
    flush_pending()
    finalize_done()


def build_nc():
    import concourse.mybir as mybir
    import concourse.tile as tile
    from concourse import bacc
    from concourse.masks import make_identity

    f32 = mybir.dt.float32
    bf16 = mybir.dt.bfloat16
    OP = mybir.AluOpType

    nc = bacc.Bacc("TRN2", target_bir_lowering=False, debug=False,
                   enable_asserts=False, num_devices=_NCORES)
    q_d = nc.dram_tensor("q", [_HLOC, _S, _D], f32, kind="ExternalInput").ap()
    k_d = nc.dram_tensor("k", [_HLOC, _S, _D], f32, kind="ExternalInput").ap()
    v_d = nc.dram_tensor("v", [_HLOC, _S, _D], f32, kind="ExternalInput").ap()
    out_d = nc.dram_tensor("out", [_HLOC, _S, _D], f32,
                           kind="ExternalOutput").ap()

    with tile.TileContext(nc) as tc:
        import contextlib
        with contextlib.ExitStack() as ctx:
            pools = {
                "natp": ctx.enter_context(tc.tile_pool(name="natp", bufs=2)),
                "psM": ctx.enter_context(
                    tc.tile_pool(name="psM", bufs=4, space="PSUM")),
                "psP": ctx.enter_context(
                    tc.tile_pool(name="psP", bufs=2, space="PSUM")),
                "big": ctx.enter_context(tc.tile_pool(name="big", bufs=2)),
                "sm": ctx.enter_context(tc.tile_pool(name="sm", bufs=2)),
                "ptp": ctx.enter_context(tc.tile_pool(name="ptp", bufs=5)),
                "outp": ctx.enter_context(tc.tile_pool(name="outp", bufs=4)),
                "constp": ctx.enter_context(
                    tc.tile_pool(name="constp", bufs=1)),
            }
            cp = pools["constp"]
            identf = cp.tile([128, 128], f32, tag="identf")
            make_identity(nc, identf[:])
            # indall[h][b, ci*128 + kj] = 1.0 iff b == 32*h + 2*ci + kj//64.
            # Two variants so both heads' negk bands pack into one tile;
            # full 128 partitions so the mask matmul keeps K=128.
            indall = []
            for h in range(_HLOC):
                ind = cp.tile([128, _NCHUNK * 128], bf16, tag=f"indall{h}",
                              name=f"indall{h}")
                nc.gpsimd.memset(ind[:], 1.0)
                nc.gpsimd.affine_select(
                    out=ind[:], in_=ind[:], compare_op=OP.is_equal,
                    fill=0.0, base=-32 * h,
                    pattern=[[-2, _NCHUNK], [-1, 2], [0, _BLK]],
                    channel_multiplier=1,
                )
                indall.append(ind)
            # negk shared between heads: rows 0-31 h0, 32-63 h1 (written
            # by the mask chains), rows 64-127 zero.
            negk = cp.tile([128, _S], bf16, tag="negk")
            nc.gpsimd.memset(negk[64:128, :], 0.0)
            # blockind[p, j] = 1.0 iff p // 64 == j  (f32 for f32 matmul)
            blockind = cp.tile([128, 2], f32, tag="blockind")
            nc.gpsimd.memset(blockind[:], 1.0)
            nc.gpsimd.affine_select(
                out=blockind[:], in_=blockind[:], compare_op=OP.is_ge,
                fill=0.0, base=0, pattern=[[-64, 2]], channel_multiplier=1)
            nc.gpsimd.affine_select(
                out=blockind[:], in_=blockind[:], compare_op=OP.is_ge,
                fill=0.0, base=63, pattern=[[64, 2]], channel_multiplier=-1)
            # tri128[p, f] = 0 if f >= p else -BIGM
            tri128 = cp.tile([128, 128], f32, tag="tri128")
            nc.gpsimd.memset(tri128[:], 0.0)
            nc.gpsimd.affine_select(
                out=tri128[:], in_=tri128[:], compare_op=OP.is_ge,
                fill=-_BIGM, base=0, pattern=[[1, 128]],
                channel_multiplier=-1,
            )
            causal_add = cp.tile([32, 32], f32, tag="causal_add")
            nc.gpsimd.memset(causal_add[:], 0.0)
            nc.gpsimd.affine_select(
                out=causal_add[:], in_=causal_add[:], compare_op=OP.is_ge,
                fill=_NEG_BL, base=0, pattern=[[-1, 32]],
                channel_multiplier=1,
            )
            causal01 = cp.tile([32, 32], f32, tag="causal01")
            nc.gpsimd.memset(causal01[:], 1.0)
            nc.gpsimd.affine_select(
                out=causal01[:], in_=causal01[:], compare_op=OP.is_ge,
                fill=0.0, base=0, pattern=[[-1, 32]],
                channel_multiplier=1,
            )
            eye01 = cp.tile([32, 32], f32, tag="eye01")
            make_identity(nc, eye01[:])
            nshift = cp.tile([128, 1], f32, tag="nshift")
            nc.gpsimd.memset(nshift[:], -_SHIFT)
            consts = dict(identf=identf, indall=indall, tri128=tri128,
                          causal_add=causal_add, causal01=causal01,
                          eye01=eye01, nshift=nshift, blockind=blockind,
                          negk=negk)
            _emit(nc, tc, pools, consts, q_d, k_d, v_d, out_d, mybir)
    nc.compile()
    return nc


_lock = threading.Lock()
_cached_nc = None


def _get_nc():
    global _cached_nc
    with _lock:
        if _cached_nc is None:
            _cached_nc = build_nc()
    return _cached_nc


def kernel(q, k, v):
    from concourse.bass_utils import run_bass_kernel_spmd

    q = np.asarray(q, dtype=np.float32)
    k = np.asarray(k, dtype=np.float32)
    v = np.asarray(v, dtype=np.float32)
    nc = _get_nc()
    in_maps = []
    for i in range(_NCORES):
        sl = slice(i * _HLOC, (i + 1) * _HLOC)
        in_maps.append({
            "q": np.ascontiguousarray(q[0, sl]),
            "k": np.ascontiguousarray(k[0, sl]),
            "v": np.ascontiguousarray(v[0, sl]),
        })
    res = run_bass_kernel_spmd(nc, in_maps, core_ids=list(range(_NCORES)))
    out = np.concatenate([res.results[i]["out"] for i in range(_NCORES)],
                         axis=0)
    return out.reshape(_B, _H, _S, _D)


if __name__ == "__main__":
    rng = np.random.default_rng(0)
    q = rng.standard_normal((_B, _H, _S, _D), dtype=np.float32)
    k = rng.standard_normal((_B, _H, _S, _D), dtype=np.float32)
    v = rng.standard_normal((_B, _H, _S, _D), dtype=np.float32)
    o = kernel(q, k, v)
    print(o.shape, o.dtype, np.abs(o).max())


# revision 9
# speedup vs baseline: 1.2625x; 1.2625x over previous
"""AdaptiveSparseAttention Trainium2 kernel (8-core head-parallel), v2.

Problem: B=1, H=16, S=2048, D=128 fp32, causal attention with an adaptive
block mask: mean-pool Q/K per 64-block, softmax block scores, keep the
minimal top-p (0.95) set of key blocks per query block (plus diagonal).

Sharding: 2 heads per NeuronCore, fully local (no collectives).

v2 changes vs baseline (106.5us):
  - per-head DMA rings (h0 on sync HWDGE, h1 on scalar HWDGE), k first,
    then the first v quarter, then q, then the v tail - so the mask
    chain and first flash waves start ~14us instead of ~45us.
  - no DVE pre-casts of k/q: PE transposes the f32 naturals directly
    (f32 transpose -> f32 psum), the psum->SBUF copy on ScalarE does
    the bf16 cast.
  - block sums for the mask via PE matmuls against a [128,2] block
    indicator (rhs), accumulated from the f32 naturals - replaces the
    slow DVE reduces and removes the cast dependency.
  - v layout copies on DVE (gpsimd was ~0.3 elem/cy and serialized the
    whole preamble behind its queue).
  - negk for both heads packed into one [128,S] tile (h0 rows 0-31,
    h1 rows 32-63, rows 64-127 zeroed once); two indall constants
    select the right band, so the per-head [96,S] memsets are gone.
  - main loop in waves of 4 chunks ([128,1024] f32 psum tiles): one
    exp per wave amortizes ScalarE's ~352-cycle per-instruction
    overhead (exp is the second-tightest engine after PE).
  - software-pipelined PV: wave w's PV matmuls issue after wave w+1's
    QK/mask matmuls, so the tensor queue never stalls on the exp.
  - outputs staged per (head, group of 256 rows) and written with one
    128KB DMA on the head's ring.
"""

import math
import threading

import numpy as np

_B, _H, _S, _D = 1, 16, 2048, 128
_NCORES = 8
_HLOC = _H // _NCORES  # heads per core
_BLK = 64
_NB = _S // _BLK       # 32 key/query blocks
_TAU = 0.95
_SCALE = 1.0 / math.sqrt(_D)
_SHIFT = 9.0           # constant softmax shift; |scaled logits| < ~6
_BIGM = 1.0e9          # additive mask magnitude (pre-scale)
_NEG_BL = -1.0e30      # block-logit causal mask value (matches reference)

_NCHUNK = _S // 128    # 16 sequence chunks of 128
_NGRP = _S // 256      # 8 query groups of 256
_WAVE = 4              # kj chunks per LT wave


class _Head:
    pass


def _emit(nc, tc, pools, consts, q_d, k_d, v_d, out_d, mybir):
    f32 = mybir.dt.float32
    bf16 = mybir.dt.bfloat16
    AF = mybir.ActivationFunctionType
    OP = mybir.AluOpType
    AX = mybir.AxisListType

    natp = pools["natp"]
    psA = pools["psA"]
    psP = pools["psP"]
    big = pools["big"]
    sm = pools["sm"]
    ptp = pools["ptp"]
    outp = pools["outp"]

    ident = consts["ident"]
    indall = consts["indall"]
    tri128 = consts["tri128"]
    causal_add = consts["causal_add"]
    causal01 = consts["causal01"]
    eye01 = consts["eye01"]
    nshift = consts["nshift"]
    blockind = consts["blockind"]

    rings = [nc.sync, nc.scalar]

    heads = []
    for h in range(_HLOC):
        H = _Head()
        H.h = h
        H.ring = rings[h]
        H.qT = big.tile([128, _S], bf16, tag="qT", name=f"qT{h}")
        H.kT = big.tile([128, _S], bf16, tag="kT", name=f"kT{h}")
        H.vb = big.tile([128, _NCHUNK * 129], bf16, tag="vb", name=f"vb{h}")
        H.vb3 = H.vb[:].rearrange("p (c x) -> p c x", x=129)
        H.negk = big.tile([128, _S], bf16, tag="negk", name=f"negk{h}")
        H.knat = natp.tile([128, _S], f32, tag="knat", name=f"knat{h}")
        H.qnat = natp.tile([128, _S], f32, tag="qnat", name=f"qnat{h}")
        H.vnat = natp.tile([128, _S], f32, tag="vnat", name=f"vnat{h}")
        H.kbn = natp.tile([128, _S], bf16, tag="kbn", name=f"kbn{h}")
        H.qbn = natp.tile([128, _S], bf16, tag="qbn", name=f"qbn{h}")
        H.bsum_sb = sm.tile([128, 64], f32, tag="bsum", name=f"bsum{h}")
        heads.append(H)

    # negk rows 32-127 are contracted against indall's zero rows; zero
    # them once so stale SBUF NaN/Inf can't poison 0*x.
    for H in heads:
        for pb in (32, 64, 96):
            nc.gpsimd.memset(H.negk[pb:pb + 32, :], 0.0)

    # ---- DMAs: per-head HWDGE ring (h0 sync, h1 scalar).  Arrival
    # order = dependency order: k half0, q half0 (mask quadrant A +
    # first waves), v chunks 0-1 (first PVs), k half1, v 2-7, q half1,
    # v tail.
    def dma_nat(H, nat, src_d, lo, n):
        H.ring.dma_start(
            nat[:, lo * 128:(lo + n) * 128].rearrange("p (c d) -> p c d",
                                                      d=128),
            src_d[H.h, lo * 128:(lo + n) * 128, :].rearrange(
                "(c p) d -> p c d", p=128))

    for H in heads:
        dma_nat(H, H.knat, k_d, 0, 8)
        dma_nat(H, H.qnat, q_d, 0, 8)
        dma_nat(H, H.vnat, v_d, 0, 2)
        dma_nat(H, H.knat, k_d, 8, 8)
        dma_nat(H, H.vnat, v_d, 2, 6)
        dma_nat(H, H.qnat, q_d, 8, 8)
        dma_nat(H, H.vnat, v_d, 8, 4)
        dma_nat(H, H.vnat, v_d, 12, 4)

    # ---- helpers ------------------------------------------------------
    def cast_half(H, tname, half):
        nat = H.knat if tname == "k" else H.qnat
        bn = H.kbn if tname == "k" else H.qbn
        lo = half * 1024
        nc.vector.tensor_copy(bn[:, lo:lo + 1024], nat[:, lo:lo + 1024])

    def transpose_half(H, tname, half, bs):
        """PE: 8 chunk transposes into a bf16 psum pack + 8 block-sum
        matmuls vs blockind into bs (psum).  Returns the pack."""
        bn = H.kbn if tname == "k" else H.qbn
        base = H.h * 32 + (0 if tname == "k" else 16)
        pack = psP.tile([128, 1024], bf16, tag="lt",
                        name=f"p{H.h}{tname}{half}")
        for j in range(8):
            c = half * 8 + j
            nc.tensor.transpose(pack[:, j * 128:(j + 1) * 128],
                                bn[:, c * 128:(c + 1) * 128], ident[:])
            nc.tensor.matmul(bs[:, base + 2 * j:base + 2 * j + 2],
                            bn[:, c * 128:(c + 1) * 128],
                            blockind[:], start=True, stop=True)
        return pack

    def pack_copy(H, tname, half, pack):
        dst = H.kT if tname == "k" else H.qT
        lo = half * 1024
        nc.vector.tensor_copy(dst[:, lo:lo + 1024], pack[:])

    def bsum_copy(H, half, bs):
        # bs cols: h*32 + {k:0,q:16} + 2j  ->  bsum_sb cols
        # {k:0,q:32} + half*16 + j*2
        h = H.h
        for toff_bs, toff_sb in ((0, 0), (16, 32)):
            nc.scalar.copy(
                H.bsum_sb[:, toff_sb + half * 16:toff_sb + half * 16 + 16],
                bs[:, h * 32 + toff_bs:h * 32 + toff_bs + 16])

    def negk_write(H, keepT_ap, rlo, rn, clo, cn):
        nc.vector.tensor_scalar(
            H.negk[rlo:rlo + rn, clo * 64:(clo + cn) * 64].rearrange(
                "p (a b) -> p a b", b=_BLK),
            keepT_ap.unsqueeze(2).broadcast_to((rn, cn, _BLK)),
            1.0, _BIGM, op0=OP.subtract, op1=OP.mult)

    def chain(H, part):
        """Block-score top-p keep mask.  part 'A': quadrant qb 0-15 x
        kb 0-15 (only needs the first k/q halves; block scores are
        shift-invariant so smooth_k's centering cancels and kb 16-31
        are not needed) -> negk cols 0-1023.  part 'B': full 32x32,
        writes only qb cols 16-31 -> negk cols 1024-2047."""
        h = H.h
        n = 16 if part == "A" else 32
        qbT = H.bsum_sb[:, 32:32 + n]
        kbT = H.bsum_sb[:, 0:n]
        blp = psA.tile([n, n], f32, tag="acc", name=f"blp{part}{h}")
        nc.tensor.matmul(blp[:], qbT, kbT, start=True, stop=True)
        bl = sm.tile([n, n], f32, tag=f"bl{part}", name=f"bl{part}{h}")
        nc.vector.scalar_tensor_tensor(
            bl[:], blp[:], _SCALE / float(_BLK * _BLK),
            causal_add[0:n, 0:n], op0=OP.mult, op1=OP.add)
        mx = sm.tile([n, 1], f32, tag=f"mx{part}", name=f"mx{part}{h}")
        nc.vector.reduce_max(mx[:], bl[:], axis=AX.X)
        nmx = sm.tile([n, 1], f32, tag=f"nmx{part}", name=f"nmx{part}{h}")
        nc.vector.tensor_scalar_mul(nmx[:], mx[:], -1.0)
        bp = sm.tile([n, n], f32, tag=f"bp{part}", name=f"bp{part}{h}")
        rs = sm.tile([n, 1], f32, tag=f"rs{part}", name=f"rs{part}{h}")
        nc.scalar.activation(bp[:], bl[:], AF.Exp, bias=nmx[:], scale=1.0,
                             accum_out=rs[:])
        taurs = sm.tile([n, 1], f32, tag=f"ta{part}", name=f"ta{part}{h}")
        nc.vector.tensor_scalar_mul(taurs[:], rs[:], _TAU)
        a_ap = bp[:].unsqueeze(1).broadcast_to((n, n, n))
        b_ap = bp[:].unsqueeze(2).broadcast_to((n, n, n))
        gt = sm.tile([n, n * n], f32, tag=f"gt{part}", name=f"gt{part}{h}")
        gt3 = gt[:].rearrange("p (a b) -> p a b", a=n)
        nc.vector.tensor_tensor(gt3, a_ap, b_ap, op=OP.is_gt)
        pr = sm.tile([n, n * n], f32, tag=f"pr{part}", name=f"pr{part}{h}")
        pr3 = pr[:].rearrange("p (a b) -> p a b", a=n)
        nc.vector.tensor_tensor(pr3, gt3, a_ap, op=OP.mult)
        tt = sm.tile([n, n], f32, tag=f"tt{part}", name=f"tt{part}{h}")
        nc.vector.reduce_sum(tt[:], pr3, axis=AX.X)
        keep = sm.tile([32, 32], f32, tag=f"kp{part}", name=f"kp{part}{h}")
        if part == "A":
            nc.vector.memset(keep[:], 0.0)
        nc.vector.scalar_tensor_tensor(
            keep[0:n, 0:n], tt[:], taurs[:], causal01[0:n, 0:n],
            op0=OP.is_lt, op1=OP.mult)
        nc.vector.tensor_tensor(keep[0:n, 0:n], keep[0:n, 0:n],
                                eye01[0:n, 0:n], op=OP.max)
        keepT = sm.tile([32, 32], f32, tag=f"kT{part}", name=f"kT{part}{h}")
        nc.vector.transpose(keepT[:], keep[:])
        if part == "A":
            # keep rows 16-31 are memset 0, so the transpose gives
            # keep=0 (-> -BIGM) for kb 16-31 over qb 0-15: causal.
            negk_write(H, keepT[:, 0:16], 0, 32, 0, 16)
        else:
            negk_write(H, keepT[:, 16:32], 0, 32, 16, 16)

    def vb_piece(H, lo, n):
        nc.gpsimd.tensor_copy(
            H.vb3[:, lo:lo + n, 0:128],
            H.vnat[:, lo * 128:(lo + n) * 128].rearrange(
                "p (c d) -> p c d", d=128))
        nc.gpsimd.memset(H.vb3[:, lo:lo + n, 128], 1.0)

    # ---- preamble: first halves -> quadrant-A mask -> main loop ------
    bsA = psA.tile([128, 64], f32, tag="acc", name="bsA")
    for H in heads:
        cast_half(H, "k", 0)
    for H in heads:
        cast_half(H, "q", 0)
    packs = {}
    for H in heads:
        packs[(H.h, "k", 0)] = transpose_half(H, "k", 0, bsA)
    for H in heads:
        packs[(H.h, "q", 0)] = transpose_half(H, "q", 0, bsA)
    for H in heads:
        pack_copy(H, "k", 0, packs[(H.h, "k", 0)])
    for H in heads:
        bsum_copy(H, 0, bsA)
    for H in heads:
        pack_copy(H, "q", 0, packs[(H.h, "q", 0)])
        chain(H, "A")
    for H in heads:
        vb_piece(H, 0, 2)

    # ---- main flash loop ---------------------------------------------
    pending = []
    done_groups = []

    def flush_pending():
        for H, g, w0, wn, ptw, acc in pending:
            nchunks = 2 * g + 2
            for ci in range(w0, w0 + wn):
                for t in range(max(2 * g, ci), 2 * g + 2):
                    nc.tensor.matmul(
                        acc[t - 2 * g][:],
                        ptw[:, (ci - w0) * 256 + (t - 2 * g) * 128:
                            (ci - w0) * 256 + (t - 2 * g) * 128 + 128],
                        H.vb3[:, ci, :],
                        start=(ci == 0), stop=(ci == t))
            if w0 + wn == nchunks:
                done_groups.append((H, g, acc))
        pending.clear()

    def finalize_done():
        for H, g, acc in done_groups:
            h = H.h
            qlo = g * 256
            rden0 = sm.tile([128, 1], f32, tag="rden", name=f"rd0_{h}_{g}")
            rden1 = sm.tile([128, 1], f32, tag="rden", name=f"rd1_{h}_{g}")
            nc.vector.reciprocal(rden0[:], acc[0][:, 128:129])
            nc.vector.reciprocal(rden1[:], acc[1][:, 128:129])
            o = outp.tile([128, 256], f32, tag="o", name=f"o{h}_{g}")
            nc.vector.tensor_scalar_mul(o[:, 0:128], acc[0][:, 0:128],
                                        rden0[:])
            nc.vector.tensor_scalar_mul(o[:, 128:256], acc[1][:, 0:128],
                                        rden1[:])
            H.ring.dma_start(
                out_d[h, qlo:qlo + 256, :].rearrange("(t p) d -> p t d",
                                                     p=128),
                o[:].rearrange("p (t d) -> p t d", d=128))
        done_groups.clear()

    bsB = None
    for g in range(_NGRP):
        nchunks = 2 * g + 2
        for H in heads:
            H.acc = [psA.tile([128, 129], f32, tag="acc",
                              name=f"acc{H.h}_{g}_{t}") for t in range(2)]
        for w0 in range(0, nchunks, _WAVE):
            wn = min(_WAVE, nchunks - w0)
            new_work = []
            for H in heads:
                qlo = g * 256
                ltw = psP.tile([128, 1024], f32, tag="lt",
                               name=f"lt{H.h}_{g}_{w0}")
                for ci in range(w0, w0 + wn):
                    sl = ltw[:, (ci - w0) * 256:(ci - w0) * 256 + 256]
                    nc.tensor.matmul(sl, H.kT[:, ci * 128:(ci + 1) * 128],
                                     H.qT[:, qlo:qlo + 256],
                                     start=True, stop=False)
                    nc.tensor.matmul(sl, indall[:, ci * 128:(ci + 1) * 128],
                                     H.negk[:, qlo:qlo + 256],
                                     start=False, stop=True)
                    if ci >= 2 * g:
                        off = (ci - w0) * 256 + (ci - 2 * g) * 128
                        nc.vector.tensor_tensor(
                            ltw[:, off:off + 128], ltw[:, off:off + 128],
                            tri128[:], op=OP.add)
                ptw = ptp.tile([128, 1024], bf16, tag="pt",
                               name=f"pt{H.h}_{g}_{w0}")
                nc.scalar.activation(ptw[:, 0:wn * 256], ltw[:, 0:wn * 256],
                                     AF.Exp, bias=nshift[:], scale=_SCALE)
                new_work.append((H, g, w0, wn, ptw, H.acc))
            flush_pending()
            pending.extend(new_work)
            finalize_done()
        # late-data work interleaved so its engine-queue position
        # matches data arrival (k half1 after g0, q half1 after g1,
        # full chain B after g2, v pieces throughout)
        if g == 0:
            bsB = psP.tile([128, 64], f32, tag="lt", name="bsB")
            for H in heads:
                cast_half(H, "k", 1)
            for H in heads:
                packs[(H.h, "k", 1)] = transpose_half(H, "k", 1, bsB)
                pack_copy(H, "k", 1, packs[(H.h, "k", 1)])
            for H in heads:
                vb_piece(H, 2, 6)
        elif g == 1:
            for H in heads:
                cast_half(H, "q", 1)
            for H in heads:
                packs[(H.h, "q", 1)] = transpose_half(H, "q", 1, bsB)
                pack_copy(H, "q", 1, packs[(H.h, "q", 1)])
            for H in heads:
                bsum_copy(H, 1, bsB)
            for H in heads:
                vb_piece(H, 8, 4)
        elif g == 2:
            for H in heads:
                chain(H, "B")
            for H in heads:
                vb_piece(H, 12, 4)
    flush_pending()
    finalize_done()


def build_nc():
    import concourse.mybir as mybir
    import concourse.tile as tile
    from concourse import bacc
    from concourse.masks import make_identity

    f32 = mybir.dt.float32
    bf16 = mybir.dt.bfloat16
    OP = mybir.AluOpType

    nc = bacc.Bacc("TRN2", target_bir_lowering=False, debug=False,
                   enable_asserts=False, num_devices=_NCORES)
    q_d = nc.dram_tensor("q", [_HLOC, _S, _D], f32, kind="ExternalInput").ap()
    k_d = nc.dram_tensor("k", [_HLOC, _S, _D], f32, kind="ExternalInput").ap()
    v_d = nc.dram_tensor("v", [_HLOC, _S, _D], f32, kind="ExternalInput").ap()
    out_d = nc.dram_tensor("out", [_HLOC, _S, _D], f32,
                           kind="ExternalOutput").ap()

    with tile.TileContext(nc) as tc:
        import contextlib
        with contextlib.ExitStack() as ctx:
            pools = {
                "natp": ctx.enter_context(tc.tile_pool(name="natp", bufs=2)),
                "psA": ctx.enter_context(
                    tc.tile_pool(name="psA", bufs=4, space="PSUM")),
                "psP": ctx.enter_context(
                    tc.tile_pool(name="psP", bufs=2, space="PSUM")),
                "big": ctx.enter_context(tc.tile_pool(name="big", bufs=2)),
                "sm": ctx.enter_context(tc.tile_pool(name="sm", bufs=2)),
                "ptp": ctx.enter_context(tc.tile_pool(name="ptp", bufs=5)),
                "outp": ctx.enter_context(tc.tile_pool(name="outp", bufs=4)),
                "constp": ctx.enter_context(
                    tc.tile_pool(name="constp", bufs=1)),
            }
            cp = pools["constp"]
            ident = cp.tile([128, 128], bf16, tag="ident")
            make_identity(nc, ident[:])
            # blockind[p, j] = 1.0 iff p // 64 == j
            blockind = cp.tile([128, 2], bf16, tag="blockind")
            nc.gpsimd.memset(blockind[:], 1.0)
            nc.gpsimd.affine_select(
                out=blockind[:], in_=blockind[:], compare_op=OP.is_ge,
                fill=0.0, base=0, pattern=[[-64, 2]], channel_multiplier=1)
            nc.gpsimd.affine_select(
                out=blockind[:], in_=blockind[:], compare_op=OP.is_ge,
                fill=0.0, base=63, pattern=[[64, 2]], channel_multiplier=-1)
            # tri128[p, f] = 0 if f >= p else -BIGM
            tri128 = cp.tile([128, 128], f32, tag="tri128")
            nc.gpsimd.memset(tri128[:], 0.0)
            nc.gpsimd.affine_select(
                out=tri128[:], in_=tri128[:], compare_op=OP.is_ge,
                fill=-_BIGM, base=0, pattern=[[1, 128]],
                channel_multiplier=-1,
            )
            causal_add = cp.tile([32, 32], f32, tag="causal_add")
            nc.gpsimd.memset(causal_add[:], 0.0)
            nc.gpsimd.affine_select(
                out=causal_add[:], in_=causal_add[:], compare_op=OP.is_ge,
                fill=_NEG_BL, base=0, pattern=[[-1, 32]],
                channel_multiplier=1,
            )
            causal01 = cp.tile([32, 32], f32, tag="causal01")
            nc.gpsimd.memset(causal01[:], 1.0)
            nc.gpsimd.affine_select(
                out=causal01[:], in_=causal01[:], compare_op=OP.is_ge,
                fill=0.0, base=0, pattern=[[-1, 32]],
                channel_multiplier=1,
            )
            eye01 = cp.tile([32, 32], f32, tag="eye01")
            make_identity(nc, eye01[:])
            nshift = cp.tile([128, 1], f32, tag="nshift")
            nc.gpsimd.memset(nshift[:], -_SHIFT)
            # indall[b, ci*128 + kj] = 1.0 iff b == 2*ci + kj//64
            # (full 128 partitions; rows 32-127 zero so K=128 everywhere)
            indall = cp.tile([128, _NCHUNK * 128], bf16, tag="indall")
            nc.gpsimd.memset(indall[:], 1.0)
            nc.gpsimd.affine_select(
                out=indall[:], in_=indall[:], compare_op=OP.is_equal,
                fill=0.0, base=0,
                pattern=[[-2, _NCHUNK], [-1, 2], [0, _BLK]],
                channel_multiplier=1,
            )
            consts = dict(ident=ident, indall=indall, tri128=tri128,
                          causal_add=causal_add, causal01=causal01,
                          eye01=eye01, nshift=nshift, blockind=blockind)
            _emit(nc, tc, pools, consts, q_d, k_d, v_d, out_d, mybir)
    nc.compile()
    return nc


_lock = threading.Lock()
_cached_nc = None


def _get_nc():
    global _cached_nc
    with _lock:
        if _cached_nc is None:
            _cached_nc = build_nc()
    return _cached_nc


def kernel(q, k, v):
    from concourse.bass_utils import run_bass_kernel_spmd

    q = np.asarray(q, dtype=np.float32)
    k = np.asarray(k, dtype=np.float32)
    v = np.asarray(v, dtype=np.float32)
    nc = _get_nc()
    in_maps = []
    for i in range(_NCORES):
        sl = slice(i * _HLOC, (i + 1) * _HLOC)
        in_maps.append({
            "q": np.ascontiguousarray(q[0, sl]),
            "k": np.ascontiguousarray(k[0, sl]),
            "v": np.ascontiguousarray(v[0, sl]),
        })
    res = run_bass_kernel_spmd(nc, in_maps, core_ids=list(range(_NCORES)))
    out = np.concatenate([res.results[i]["out"] for i in range(_NCORES)],
                         axis=0)
    return out.reshape(_B, _H, _S, _D)


if __name__ == "__main__":
    rng = np.random.default_rng(0)
    q = rng.standard_normal((_B, _H, _S, _D), dtype=np.float32)
    k = rng.standard_normal((_B, _H, _S, _D), dtype=np.float32)
    v = rng.standard_normal((_B, _H, _S, _D), dtype=np.float32)
    o = kernel(q, k, v)
    print(o.shape, o.dtype, np.abs(o).max())
